# revision 1
# baseline (speedup 1.0000x reference)
"""Trainium2 Bass kernel for nn_DiceLoss_11038065951148.

Reference semantics: cm[t,p] += (t==p)  -> only the diagonal accumulates, so
tp[c] = #{i : pred_i == target_i == c}; fn = fp = 0 exactly.
dice = mean_{c=1..3} 2*tp/(2*tp + 1e-6); loss = balance * (1 - dice**0.75).

Kernel strategy (memory-bound streaming, data-parallel over 8 cores):
  - shard the [1, N] int32 label arrays into 8 contiguous chunks of
    N/8 = 2,097,152 elements = [128 partitions, 16384]; pred/targ are
    interleaved per partition row on the host so each tile is ONE
    contiguous DMA (the compute ISA structs have very few sync-wait slots)
  - per tile (ramped widths: small edge tiles for fast ramp/short tail,
    2 MB middle tiles for bandwidth):
      DVE: u = 4*targ + pred via one fused scalar_tensor_tensor (bf16;
           u == 5c  <=>  pred == targ == c), plus count(u==5) via
           tensor_scalar(is_equal, accum) -> class 1
      ACT: Sign passes sum(sign(u - k)); sign is exactly +/-1 (u integer,
           k half-integer), so host gets class 2 = (S9.5 - S10.5)/2 and
           class 3 = (S14.5 + N/8)/2
      Class-2 extraction is column-split per tile width (split_cols):
      edge tiles fully on DVE, mid tiles mostly on ACT with a small DVE
      slice — balancing DVE (~36 us) and ACT (~38 us) busy time under the
      DMA floor, and shrinking the near-critical ACT tail on the last tile.
    (DVE fast modes don't engage with accum_out, so ops price at 1x.)
  - one [128, 5, ntiles] f32 accumulator tile (rows 0-1 DVE, rows 2-4 ACT;
    unwritten slices stay 0 from a memset so the host formula is
    universal) is stored back in a single DMA; the host sums counts in
    float64, rounds to exact integers, applies the float32 dice formula.

Measured (serialized single-NEFF repeats, 8 cores): 47.3 us best / ~50 us
typical per kernel execution; steady-state pipelined ~44.5 us = the
~369 GB/s per-core HBM streaming floor for 16.78 MB/core of int32 labels.
"""

import os
import sys

for _p in ("/opt/trn_rl_repo", "/opt/pypackages"):
    if _p not in sys.path:
        sys.path.insert(0, _p)

import numpy as np

# Set by the last kernel() call when DICE_TRACE=1: the BassKernelResults
# (exec_time_ns etc.) from run_bass_kernel_spmd. Used by test.py only.
last_results = None

N = 16_777_216
NCORES = 8
PER_CORE = N // NCORES  # 2,097,152
P = 128
TOT = PER_CORE // P  # 16384 elements per partition per tensor
NT = 4  # tiles per tensor per core (uniform default)
W = TOT // NT  # 4096
# ramped schedule: small tiles first (compute starts early) and last
# (short tail), 4 MB tiles in the middle for bandwidth
WIDTHS_RAMP = (1024, 1024, 1024, 4096, 4096, 4096, 1024)
WIDTHS_RAMP2 = (1024, 1024, 2048, 2048, 2048, 2048, 2048, 2048, 1024, 1024)
WIDTHS_RAMP3 = (512, 512, 1024, 2048, 2048, 2048, 2048, 2048, 2048, 1024, 512, 512)
WIDTHS_RAMP5 = (1024, 1024, 4096, 4096, 2048, 2048, 1024, 1024)
WIDTHS_RAMP6 = (1024, 1024, 2048, 2048, 2048, 2048, 2048, 2048, 1024, 512, 512)
# 4 MB mid-tiles (best stream rate) with small edges; class-2 work is
# column-split inside the big tiles so both engines fit the DMA cadence
WIDTHS_RAMP7 = (1024, 1024, 4096, 4096, 4096, 1024, 1024)
WIDTHS_RAMP8 = (1024, 1024, 3072, 3072, 3072, 3072, 1024, 1024)
WIDTHS_RAMP9 = (1024, 1024, 2048, 2048, 2048, 2048, 2048, 2048, 1536, 512)
WIDTHS_RAMP10 = (1024, 2048, 2048, 2048, 2048, 2048, 2048, 2048, 1024)
WIDTHS_RAMP11 = (512, 1536, 2048, 2048, 2048, 2048, 2048, 2048, 1024, 1024)
# tiles at most this wide extract class 2 on DVE (one is_equal) instead of
# two ACT sign passes — ACT is the near-critical engine at the tail
CLS2_DVE_MAX_W = 1024


def split_cols(wd):
    """Number of leading columns whose class-2 count runs on DVE (is_equal);
    the remaining columns use the ACT sign-pair. Balances DVE (~9.1 us) and
    ACT (~9.4 us) under the 4 MB DMA cadence (~10.5 us) on big tiles."""
    if wd <= CLS2_DVE_MAX_W:
        return wd
    if wd >= 4096:
        return 512
    if wd >= 2048:
        return 256
    return 0


def build(
    nt=NT,
    w=W,
    repeat=1,
    compute=True,
    widths=None,
    serialize=False,
    dual_dge=False,
    # 5-deep input prefetch on the mid tiles: absorbs compute jitter so the
    # DMA stream never stalls on a WAR slot (measured ~1 us better than 3)
    io_bufs=5,
):
    import concourse.bacc as bacc
    import concourse.mybir as mybir
    from concourse._compat import axon_active
    from concourse.tile import TileContext, add_dep_helper

    nc = bacc.Bacc(
        "TRN2",
        target_bir_lowering=False,
        debug=not axon_active(),
        num_devices=NCORES,
        name="dice_hist",
    )
    if widths is None:
        widths = [w] * nt
    widths = list(widths)
    tot = sum(widths)
    nt = len(widths)
    offs = [sum(widths[:i]) for i in range(nt)]
    # pred and target interleaved per partition row so each tile is ONE dma
    # (one DMA-sem wait on the consuming compute op — the compute ISA structs
    # have very few sync-wait slots).
    # layout: [P, 2, tot]; tile i = columns [offs[i], offs[i]+widths[i])
    pt_d = nc.dram_tensor("pt", [P, 2, tot], mybir.dt.int32, kind="ExternalInput")
    # rows (middle axis): 0 = count(u==5) [class1]; 1 = count(u==10) over
    # each tile's leading split_cols(wd) columns (DVE); 2,3 = sum(sign(u-k))
    # for k in (9.5, 10.5) over the remaining columns (ACT); 4 = full-width
    # sum(sign(u-14.5)). Unwritten slices stay 0 from the memset, so the
    # host formula is universal: n2 = r1 + (r2-r3)/2, n3 = (r4+N)/2.
    out_d = nc.dram_tensor("out", [P, 5, nt], mybir.dt.float32, kind="ExternalOutput")

    THRESH = (9.5, 10.5, 14.5)
    n_of_width = {wd: widths.count(wd) for wd in set(widths)}

    with TileContext(nc) as tc:
        with (
            tc.tile_pool(name="io", bufs=1) as io_pool,
            tc.tile_pool(name="wk", bufs=2) as wk_pool,
            tc.tile_pool(name="acc", bufs=1) as acc_pool,
        ):
            # one accumulator tile; rows 0-1 written by DVE, rows 2-4 by ACT
            # (disjoint slices, so no cross-engine hazards)
            acc_all = acc_pool.tile([P, 5, nt], mybir.dt.float32, tag="acc")
            nc.gpsimd.memset(acc_all[:], 0.0)
            acc1 = acc_all[:, 0, :]
            acc10 = acc_all[:, 1, :]
            accs = [acc_all[:, k + 2, :] for k in range(3)]
            biases = []
            for k, th in enumerate(THRESH):
                b = acc_pool.tile([P, 1], mybir.dt.float32, tag=f"bias{k}")
                nc.gpsimd.memset(b[:], -th)
                biases.append(b)
            prev_tail = None
            for _r in range(repeat):
                tail_inst = None
                for i in range(nt):
                    wd = widths[i]
                    tile2 = io_pool.tile(
                        [P, 2, wd],
                        mybir.dt.int32,
                        tag=f"pt{wd}",
                        bufs=min(n_of_width[wd], io_bufs),
                    )
                    # optionally alternate HWDGE (sync seq) and SWDGE
                    # (gpsimd) descriptor generation so two DMA lifecycles
                    # overlap; gpsimd is otherwise idle
                    dma_eng = nc.gpsimd if (dual_dge and i % 2 == 1) else nc.sync
                    d = dma_eng.dma_start(
                        tile2[:], pt_d[:, :, offs[i] : offs[i] + wd]
                    )
                    if serialize and prev_tail is not None:
                        add_dep_helper(
                            d.ins, prev_tail, sync=True, reason="serialize repeats"
                        )
                    if not compute:
                        continue
                    p_v = tile2[:, 0, :]
                    t_v = tile2[:, 1, :]
                    # u = 4*t + p in one fused op; u == 5c  <=>  p == t == c
                    u = wk_pool.tile([P, wd], mybir.dt.bfloat16, tag=f"u{wd}")
                    nc.vector.scalar_tensor_tensor(
                        out=u[:],
                        in0=t_v,
                        scalar=4.0,
                        in1=p_v,
                        op0=mybir.AluOpType.mult,
                        op1=mybir.AluOpType.add,
                    )
                    # class 1 on DVE: count(u == 5)
                    dm1 = wk_pool.tile([P, wd], mybir.dt.bfloat16, tag=f"dm1{wd}")
                    nc.vector.tensor_scalar(
                        out=dm1[:],
                        in0=u[:],
                        scalar1=5.0,
                        scalar2=None,
                        op0=mybir.AluOpType.is_equal,
                        op1=mybir.AluOpType.add,
                        accum_out=acc1[:, i : i + 1],
                    )
                    # class 2 split: leading ws columns counted on DVE,
                    # the rest via the ACT sign pair
                    ws = split_cols(wd)
                    if ws > 0:
                        dm2 = wk_pool.tile(
                            [P, wd], mybir.dt.bfloat16, tag=f"dm1{wd}"
                        )
                        nc.vector.tensor_scalar(
                            out=dm2[:, :ws],
                            in0=u[:, :ws],
                            scalar1=10.0,
                            scalar2=None,
                            op0=mybir.AluOpType.is_equal,
                            op1=mybir.AluOpType.add,
                            accum_out=acc10[:, i : i + 1],
                        )
                    # threshold step sums on ACT: sign(u - k) = +/-1 exactly
                    # (u integer, k half-integer), so sum = 2*count(u > k) - n
                    # over whatever column range the op covers.
                    for k in (0, 1, 2):
                        cols = slice(None) if k == 2 else slice(ws, wd)
                        if k != 2 and ws >= wd:
                            continue
                        dmk = wk_pool.tile([P, wd], mybir.dt.bfloat16, tag=f"dmA{wd}")
                        a = nc.scalar.activation(
                            out=dmk[:, cols],
                            in_=u[:, cols],
                            func=mybir.ActivationFunctionType.Sign,
                            bias=biases[k][:],
                            scale=1.0,
                            accum_out=accs[k][:, i : i + 1],
                        )
                        tail_inst = a.ins
                prev_tail = tail_inst
            if compute:
                nc.sync.dma_start(out_d[:], acc_all[:])
            else:
                nc.gpsimd.dma_start(out_d[:], tile2[:, 0, : 4 * nt])
    nc.compile()
    return nc


DEFAULT_WIDTHS = WIDTHS_RAMP2

_nc_cache = None


def _get_nc():
    global _nc_cache
    if _nc_cache is None:
        _nc_cache = build(widths=DEFAULT_WIDTHS)
    return _nc_cache


def unpack_counts(out_arr, widths=None):
    """Per-core [P, 5, nt] device output -> (n1, n2, n3) float64 counts.

    Universal: unwritten accumulator slices are 0, so per tile
    n2 = count10_dve + (signA - signB)/2 holds for every split choice."""
    a = np.asarray(out_arr, dtype=np.float64).sum(axis=(0, 2))  # [5]
    n1 = a[0]
    n2 = a[1] + (a[2] - a[3]) / 2.0
    n3 = (a[4] + P * (TOT if widths is None else sum(widths))) / 2.0
    return n1, n2, n3


def _dice_from_counts(counts, balance, num_classes):
    # counts: float64 [4]; replicate the reference float32 arithmetic
    tp = counts.astype(np.float32)
    denom = (np.float32(2.0) * tp + np.float32(1e-6)).astype(np.float32)
    dice_per_class = (np.float32(2.0) * tp / denom).astype(np.float32)
    dice = np.float32(dice_per_class[1:].sum()) / np.float32(num_classes - 1)
    loss = np.float32(balance) * (np.float32(1.0) - dice ** np.float32(0.75))
    return np.float32(loss)


def kernel(**inputs):
    pred = np.ascontiguousarray(np.asarray(inputs["pred_labels"], dtype=np.int32))
    targ = np.ascontiguousarray(np.asarray(inputs["target_labels"], dtype=np.int32))
    balance = np.float32(np.asarray(inputs.get("balance", 1.0)))
    num_classes = int(np.asarray(inputs.get("num_classes", 4)))

    from concourse.bass_utils import run_bass_kernel_spmd

    nc = _get_nc()
    pred_sh = pred.reshape(NCORES, P, 1, TOT)
    targ_sh = targ.reshape(NCORES, P, 1, TOT)
    # interleave per partition row: [NCORES, P, 2, TOT]
    pt = np.concatenate([pred_sh, targ_sh], axis=2)
    in_maps = [{"pt": pt[i]} for i in range(NCORES)]
    trace = os.environ.get("DICE_TRACE", "") == "1"
    res = run_bass_kernel_spmd(
        nc, in_maps, core_ids=list(range(NCORES)), trace=trace
    )
    global last_results
    last_results = res

    counts = np.zeros(4, dtype=np.float64)
    for r in res.results:
        n1, n2, n3 = unpack_counts(r["out"], DEFAULT_WIDTHS)
        counts[1] += n1
        counts[2] += n2
        counts[3] += n3
    counts = np.rint(counts)
    return _dice_from_counts(counts, balance, num_classes)

